# revision 15
# baseline (speedup 1.0000x reference)
"""CircleLoss (nn_CircleLoss_55482387529741) Trainium2 Bass kernel, v2.

Math (B=8192, D=128, m=0.25, g=256):
  ahat = l2norm(A) rows, bhat = l2norm(B) rows, s_ij = ahat_i . bhat_j
  exp(logit_neg) = exp(max(16*s, 4)^2 - 16)   (identity; cold s<=m -> exp(0)=1)
  lse_pos_ii = (w-12)(w-4), w = min(16*s_ii, 12)
  loss_i = softplus(lse_pos_i + log(sum_{j!=i} exp(logit_neg_ij)));  out = mean

Distribution: A rows sharded 8 x 1024.  Core layout is FLIPPED vs v1:
partitions = a-rows (8 tiles of 128), free = all 8192 b-cols.  The row
reduction sum_j exp(...) then rides the ACT engine's fused accum_out,
eliminating the ones-matmul reduction pass entirely.  B is rotated per
core on host so each a-tile t's diagonal lands at b-cols [t*128, t*128+128).

Per core engine pipeline (steady state):
  PE    : sim matmuls  aT16[t] (stationary, fp16) x bT16 chunks (moving, fp16)
  DVE   : z = sq(max(r * invb_j, 4))  custom 2-src op, PSUM -> SBUF f32
  ACT   : e = exp(z - 16) with accum_out = per-row sums  (one [128,8192] op / tile)
  GPSIMD: b sumsq squares, invb partition-broadcast (prep), off hot path
Norms: ssb via ones-matmul on PE (column sums of bsq) -> DRAM round-trip
reshape -> rsqrt via ln/exp -> broadcast [1,8192] -> [128,8192] (gpsimd).
Diagonal excluded by subtracting exp(logit_neg(s_ii)) from the row sum.
"""

import sys

for _p in ("/opt/trn_rl_repo",):
    if _p not in sys.path:
        sys.path.append(_p)

import numpy as np

import concourse.bass as bass
from concourse import bacc
import concourse.mybir as mybir
import concourse.tile as tile
from concourse.bass_utils import run_bass_kernel_spmd
from concourse.masks import make_identity

F32 = mybir.dt.float32
F16 = mybir.dt.float16
BF16 = mybir.dt.bfloat16
AF = mybir.ActivationFunctionType
OP = mybir.AluOpType

B = 8192
D = 128
NCORES = 8
MPC = B // NCORES  # 1024 a-rows per core
NT = MPC // 128  # 8 a-tiles
NK = B // 1024  # 8 b-chunks of 1024
CHUNK = 2048  # z-op granularity (psum tile free size)
NC = B // CHUNK  # 4 psum chunks per a-tile row
LN16 = float(np.log(16.0))

_cache = {}

import os


def _dbg(flag):
    return flag in os.environ.get("KDBG", "").split(",")


def _get_custom_ops():
    """Register (once) the custom DVE ops:
    CIRCLE_CLAMP_SQ : out = sq(maxx(in0*s0, s1))   (scalar-scale variant)
    CIRCLE_CLAMP_SQ2: out = sq(maxx(in0*in1, s1))  (2-src: in1 = invb bcast)
    """
    from concourse import dve_ops
    from concourse.dve_spec import Spec, Src0, Src1, C0, C1, maxx, sq, lower
    from concourse.dve_spec import _has_src1 as has_src1
    from concourse.dve_uop import DveOpSpec

    def _register(name, spec):
        for o in dve_ops.OPS:
            if o.name == name:
                return o
        opcode = dve_ops._CUSTOM_DVE_ROW_BASE + len(dve_ops.OPS)
        assert opcode < 0x20
        shas = {}
        for ver in ("v3", "v4"):
            try:
                shas[ver] = DveOpSpec(
                    name=name,
                    opcode=opcode,
                    uops=lower(spec, ver=ver),
                    rd1_en=has_src1(spec),
                ).sha(ver)
            except Exception:
                pass
        op = dve_ops.DveOp(name, spec, subdim=False, uops_sha=shas)
        dve_ops.OPS.append(op)
        dve_ops.CUSTOM_DVE_SPECS[name] = spec
        dve_ops._SUB_OPCODE_FOR_NAME[name] = opcode
        return op

    def _ref1(in0, in1, s0, s1, imm2):
        return np.square(
            np.maximum(in0.astype(np.float32) * np.float32(s0), np.float32(s1))
        ).astype(np.float32)

    def _ref2(in0, in1, s0, s1, imm2):
        return np.square(
            np.maximum(in0.astype(np.float32) * in1.astype(np.float32), np.float32(s1))
        ).astype(np.float32)

    def _ref3(in0, in1, s0, s1, imm2):
        return (in0.astype(np.float32) * np.float32(s0)).astype(np.float32)

    op1 = _register(
        "CIRCLE_CLAMP_SQ", Spec(body=sq(maxx(Src0 * C0, C1)), reference=_ref1)
    )
    op2 = _register(
        "CIRCLE_CLAMP_SQ2", Spec(body=sq(maxx(Src0 * Src1, C1)), reference=_ref2)
    )
    op3 = _register("CIRCLE_MUL", Spec(body=Src0 * C0, reference=_ref3))
    return op1, op2, op3


def _build():
    if "nc" in _cache:
        return _cache["nc"]
    op_sq, op_sq2, op_mul = _get_custom_ops()
    from concourse.dve_ops import TENSOR_TENSOR_REDUCE as op_ttr
    nc = bacc.Bacc("TRN2", target_bir_lowering=False)

    a_in = nc.declare_dram_parameter("a_shard", [MPC, D], F32, isOutput=False)
    bT_in = nc.declare_dram_parameter("bT16", [D, B], F16, isOutput=False)
    out = nc.declare_dram_parameter("losses", [MPC], F32, isOutput=True)
    ssb_scr = nc.dram_tensor("ssb_scratch", [B], F32)
    ivb_scr = nc.dram_tensor("ivb_scratch", [B], F32)
    out_pm = out.rearrange("(m p) -> p m", p=128)  # [128, 8] view

    with tile.TileContext(nc) as tc:
        with (
            tc.tile_pool(name="consts", bufs=1) as consts,
            tc.tile_pool(name="big", bufs=1) as big,
            tc.tile_pool(name="bsqp", bufs=2) as bsqp,
            tc.tile_pool(name="norm", bufs=2) as normp,
            tc.tile_pool(name="flat", bufs=2) as flatp,
            tc.tile_pool(name="zpool", bufs=2) as zpool,
            tc.tile_pool(name="stats", bufs=1) as stats,
            tc.tile_pool(name="psim", bufs=2, space="PSUM") as psim,
        ):
            # ---- constants ----
            eye = consts.tile([128, 128], F32, tag="eye")
            make_identity(nc, eye)
            ones16 = consts.tile([128, 1], F16, tag="ones16")
            nc.vector.memset(ones16, 1.0)
            onecol = consts.tile([1, 128], F32, tag="onecol")
            nc.vector.memset(onecol, 1.0)
            b_ln16 = consts.tile([128, 1], F32, tag="b_ln16")
            nc.vector.memset(b_ln16, LN16)
            b_m16 = consts.tile([128, 1], F32, tag="b_m16")
            nc.vector.memset(b_m16, -16.0)

            # ---- persistent tensors ----
            bT16 = big.tile([128, B], F16, tag="bT16")  # raw b^T (fp16)
            ivbb = big.tile([128, B], F16, tag="ivbb")  # bcast 1/||b_j||
            aT16 = big.tile([128, MPC], F16, tag="aT16")  # (a*16/||a||)^T
            escr = big.tile([128, B], BF16, tag="escr")  # exp dump (never read)
            S = stats.tile([128, NT], F32, tag="S")  # sum_j exp per a-row
            rdraw = stats.tile([128, NT], F32, tag="rdraw")  # raw diag r
            ivb0 = stats.tile([128, NK], F32, tag="ivb0")  # invb f32, chunk0 cols
            dscr = stats.tile([128, 128], F32, tag="dscr")  # TTR dummy out

            # ---- a prep: sumsq -> inva16 -> scale(f32) -> transpose -> fp16 ----
            a128 = big.tile([128, NT, D], F32, tag="a128")
            nc.sync.dma_start(out=a128, in_=a_in.rearrange("(t p) d -> p t d", p=128))
            ssa = stats.tile([128, NT], F32, tag="ssa")
            for t in range(NT):
                nc.vector._custom_dve(
                    op_ttr,
                    out=dscr,
                    in0=a128[:, t, :],
                    in1=a128[:, t, :],
                    s0=0.0,
                    s1=1.0,
                    accum_out=ssa[:, t:t + 1],
                )
            lssa = stats.tile([128, NT], F32, tag="lssa")
            nc.scalar.activation(out=lssa, in_=ssa, func=AF.Ln)
            inva16 = stats.tile([128, NT], F32, tag="inva16")
            nc.scalar.activation(
                out=inva16, in_=lssa, func=AF.Exp, scale=-0.5, bias=b_ln16
            )
            a16 = big.tile([128, NT, D], F32, tag="a16")
            for t in range(NT):
                nc.vector._custom_dve(
                    op_mul,
                    out=a16[:, t, :],
                    in0=a128[:, t, :],
                    s0=inva16[:, t:t + 1],
                )
            for q in range(2):  # two psum batches of 4 transposes
                pt = psim.tile([128, CHUNK], F32, tag="sim")
                for j in range(4):
                    nc.tensor.transpose(
                        pt[:, j * 128:(j + 1) * 128], a16[:, q * 4 + j, :], eye
                    )
                nc.scalar.copy(out=aT16[:, q * 512:(q + 1) * 512], in_=pt[:, 0:512])

            # ---- b prep per 1024-col chunk k ----
            for k in range(NK):
                K = k * 1024
                nc.gpsimd.dma_start(out=bT16[:, K:K + 1024], in_=bT_in[:, K:K + 1024])
                bsq = bsqp.tile([128, 1024], F16, tag="bsq")
                nc.vector.tensor_mul(bsq, bT16[:, K:K + 1024], bT16[:, K:K + 1024])
                pt = psim.tile([128, CHUNK], F32, tag="sim")
                for h in range(2):
                    nc.tensor.matmul(
                        pt[0:1, h * 512:(h + 1) * 512],
                        ones16,
                        bsq[:, h * 512:(h + 1) * 512],
                        start=True,
                        stop=True,
                    )
                ssbf = flatp.tile([1, 1024], F32, tag="ssbf")
                nc.scalar.copy(out=ssbf, in_=pt[0:1, 0:1024])
                nc.sync.dma_start(out=ssb_scr[K:K + 1024], in_=ssbf)
                ssb128 = normp.tile([128, 8], F32, tag="ssb128")
                nc.sync.dma_start(
                    out=ssb128,
                    in_=ssb_scr[K:K + 1024].rearrange("(m p) -> p m", p=128),
                )
                lssb = normp.tile([128, 8], F32, tag="lssb")
                nc.scalar.activation(out=lssb, in_=ssb128, func=AF.Ln)
                ivb16 = normp.tile([128, 8], F32, tag="ivb16")
                nc.scalar.activation(out=ivb16, in_=lssb, func=AF.Exp, scale=-0.5)
                if k == 0:
                    nc.scalar.activation(out=ivb0, in_=lssb, func=AF.Exp, scale=-0.5)
                nc.sync.dma_start(
                    out=ivb_scr[K:K + 1024].rearrange("(m p) -> p m", p=128),
                    in_=ivb16,
                )
                ivbf = flatp.tile([1, 1024], F32, tag="ivbf")
                nc.sync.dma_start(out=ivbf, in_=ivb_scr[K:K + 1024])
                if _dbg("gpsbcast"):
                    nc.gpsimd.partition_broadcast(ivbb[:, K:K + 1024], ivbf)
                else:
                    ptb = psim.tile([128, CHUNK], F32, tag="sim")
                    for h in range(2):
                        nc.tensor.matmul(
                            ptb[:, h * 512:(h + 1) * 512],
                            onecol,
                            ivbf[:, h * 512:(h + 1) * 512],
                            start=True,
                            stop=True,
                        )
                    nc.scalar.copy(out=ivbb[:, K:K + 1024], in_=ptb[:, 0:1024])

            # ---- main: per a-tile t, full [128 x 8192] row ----
            for t in range(0 if _dbg("skipmain") else NT):
                zb = zpool.tile([128, B], F32, tag="z")
                for c in range(NC):
                    cc = c * CHUNK
                    pt = psim.tile([128, CHUNK], F32, tag="sim")
                    for h in range(CHUNK // 512):
                        nc.tensor.matmul(
                            pt[:, h * 512:(h + 1) * 512],
                            aT16[:, t * 128:(t + 1) * 128],
                            bT16[:, cc + h * 512:cc + (h + 1) * 512],
                            start=True,
                            stop=True,
                        )
                    if c == 0:
                        nc.vector._custom_dve(
                            op_ttr,
                            out=dscr,
                            in0=pt[:, t * 128:(t + 1) * 128],
                            in1=eye,
                            s0=0.0,
                            s1=1.0,
                            accum_out=rdraw[:, t:t + 1],
                        )
                    if _dbg("noop2"):
                        nc.vector._custom_dve(
                            op_sq,
                            out=zb[:, cc:cc + CHUNK],
                            in0=pt,
                            s0=0.0884,
                            s1=4.0,
                        )
                    else:
                        nc.vector._custom_dve(
                            op_sq2,
                            out=zb[:, cc:cc + CHUNK],
                            in0=pt,
                            in1=ivbb[:, cc:cc + CHUNK],
                            s1=4.0,
                        )
                if _dbg("noaccum"):
                    nc.scalar.activation(out=escr, in_=zb, func=AF.Exp, bias=b_m16)
                    nc.vector.tensor_reduce(
                        out=S[:, t:t + 1],
                        in_=escr,
                        axis=mybir.AxisListType.X,
                        op=OP.add,
                    )
                else:
                    nc.scalar.activation(
                        out=escr,
                        in_=zb,
                        func=AF.Exp,
                        bias=b_m16,
                        accum_out=S[:, t:t + 1],
                    )

            # ---- epilogue: [128, 8] ----
            if _dbg("skipmain"):
                nc.vector.memset(S, 1.0)
                nc.vector.memset(rdraw, 1.0)
            rd16 = stats.tile([128, NT], F32, tag="rd16")  # 16 * s_ii
            nc.vector.tensor_mul(rd16, rdraw, ivb0)
            zd = stats.tile([128, NT], F32, tag="zd")
            nc.vector._custom_dve(op_sq, out=zd, in0=rd16, s0=1.0, s1=4.0)
            ed = stats.tile([128, NT], F32, tag="ed")
            nc.scalar.activation(out=ed, in_=zd, func=AF.Exp, bias=b_m16)
            S2 = stats.tile([128, NT], F32, tag="S2")
            nc.vector.scalar_tensor_tensor(
                out=S2, in0=S, scalar=0.0, in1=ed, op0=OP.add, op1=OP.subtract
            )
            lse = stats.tile([128, NT], F32, tag="lse")
            nc.scalar.activation(out=lse, in_=S2, func=AF.Ln)
            w = stats.tile([128, NT], F32, tag="w")
            nc.vector.scalar_tensor_tensor(
                out=w, in0=rd16, scalar=12.0, in1=rd16, op0=OP.min, op1=OP.min
            )
            lpr = stats.tile([128, NT], F32, tag="lpr")
            nc.vector.scalar_tensor_tensor(
                out=lpr, in0=w, scalar=16.0, in1=w, op0=OP.subtract, op1=OP.mult
            )
            tt = stats.tile([128, NT], F32, tag="tt")
            nc.vector.scalar_tensor_tensor(
                out=tt, in0=lpr, scalar=48.0, in1=lse, op0=OP.add, op1=OP.add
            )
            abst = stats.tile([128, NT], F32, tag="abst")
            nc.scalar.activation(out=abst, in_=tt, func=AF.Abs)
            u = stats.tile([128, NT], F32, tag="u")
            nc.scalar.activation(out=u, in_=abst, func=AF.Exp, scale=-1.0)
            v = stats.tile([128, NT], F32, tag="v")
            nc.scalar.activation(out=v, in_=u, func=AF.Ln, bias=1.0)
            loss = stats.tile([128, NT], F32, tag="loss")
            nc.vector.scalar_tensor_tensor(
                out=loss, in0=tt, scalar=0.0, in1=v, op0=OP.max, op1=OP.add
            )
            nc.sync.dma_start(out=out_pm, in_=loss)

    nc.finalize()
    _cache["nc"] = nc
    return nc


def _make_in_maps(A: np.ndarray, Bm: np.ndarray) -> list[dict]:
    in_maps = []
    for c in range(NCORES):
        br = np.roll(Bm, -MPC * c, axis=0)
        in_maps.append(
            {
                "a_shard": np.ascontiguousarray(A[MPC * c:MPC * (c + 1)]),
                "bT16": np.ascontiguousarray(br.T).astype(np.float16),
            }
        )
    return in_maps


def kernel(embeddings_a: np.ndarray, embeddings_b: np.ndarray) -> np.ndarray:
    nc = _build()
    A = np.ascontiguousarray(embeddings_a, dtype=np.float32)
    Bm = np.ascontiguousarray(embeddings_b, dtype=np.float32)
    in_maps = _make_in_maps(A, Bm)
    res = run_bass_kernel_spmd(nc, in_maps, list(range(NCORES))).results
    losses = np.concatenate([res[c]["losses"] for c in range(NCORES)])
    return np.float32(np.mean(losses.astype(np.float64)))


# revision 17
# speedup vs baseline: 1.5634x; 1.5634x over previous
"""CircleLoss (nn_CircleLoss_55482387529741) Trainium2 Bass kernel, v2.

Math (B=8192, D=128, m=0.25, g=256):
  ahat = l2norm(A) rows, bhat = l2norm(B) rows, s_ij = ahat_i . bhat_j
  exp(logit_neg) = exp(max(16*s, 4)^2 - 16)   (identity; cold s<=m -> exp(0)=1)
  lse_pos_ii = (w-12)(w-4), w = min(16*s_ii, 12)
  loss_i = softplus(lse_pos_i + log(sum_{j!=i} exp(logit_neg_ij)));  out = mean

Distribution: A rows sharded 8 x 1024.  Core layout is FLIPPED vs v1:
partitions = a-rows (8 tiles of 128), free = all 8192 b-cols.  The row
reduction sum_j exp(...) then rides the ACT engine's fused accum_out,
eliminating the ones-matmul reduction pass entirely.  B is rotated per
core on host so each a-tile t's diagonal lands at b-cols [t*128, t*128+128).

Per core engine pipeline (steady state):
  PE    : sim matmuls  aT16[t] (stationary, fp16) x bT16 chunks (moving, fp16)
  DVE   : z = sq(max(r * invb_j, 4))  custom 2-src op, PSUM -> SBUF f32
  ACT   : e = exp(z - 16) with accum_out = per-row sums  (one [128,8192] op / tile)
  GPSIMD: b sumsq squares, invb partition-broadcast (prep), off hot path
Norms: ssb via ones-matmul on PE (column sums of bsq) -> DRAM round-trip
reshape -> rsqrt via ln/exp -> broadcast [1,8192] -> [128,8192] (gpsimd).
Diagonal excluded by subtracting exp(logit_neg(s_ii)) from the row sum.
"""

import sys

for _p in ("/opt/trn_rl_repo",):
    if _p not in sys.path:
        sys.path.append(_p)

import numpy as np

import concourse.bass as bass
from concourse import bacc
import concourse.mybir as mybir
import concourse.tile as tile
from concourse.bass_utils import run_bass_kernel_spmd
from concourse.masks import make_identity

F32 = mybir.dt.float32
F16 = mybir.dt.float16
BF16 = mybir.dt.bfloat16
AF = mybir.ActivationFunctionType
OP = mybir.AluOpType

B = 8192
D = 128
NCORES = 8
MPC = B // NCORES  # 1024 a-rows per core
NT = MPC // 128  # 8 a-tiles
NK = B // 1024  # 8 b-chunks of 1024
CHUNK = 2048  # z-op granularity (psum tile free size)
NC = B // CHUNK  # 4 psum chunks per a-tile row
LN16 = float(np.log(16.0))

_cache = {}

import os


def _dbg(flag):
    return flag in os.environ.get("KDBG", "").split(",")


def _get_custom_ops():
    """Register (once) the custom DVE ops:
    CIRCLE_CLAMP_SQ : out = sq(maxx(in0*s0, s1))   (scalar-scale variant)
    CIRCLE_CLAMP_SQ2: out = sq(maxx(in0*in1, s1))  (2-src: in1 = invb bcast)
    """
    from concourse import dve_ops
    from concourse.dve_spec import Spec, Src0, Src1, C0, C1, maxx, sq, lower
    from concourse.dve_spec import _has_src1 as has_src1
    from concourse.dve_uop import DveOpSpec

    def _register(name, spec):
        for o in dve_ops.OPS:
            if o.name == name:
                return o
        opcode = dve_ops._CUSTOM_DVE_ROW_BASE + len(dve_ops.OPS)
        assert opcode < 0x20
        shas = {}
        for ver in ("v3", "v4"):
            try:
                shas[ver] = DveOpSpec(
                    name=name,
                    opcode=opcode,
                    uops=lower(spec, ver=ver),
                    rd1_en=has_src1(spec),
                ).sha(ver)
            except Exception:
                pass
        op = dve_ops.DveOp(name, spec, subdim=False, uops_sha=shas)
        dve_ops.OPS.append(op)
        dve_ops.CUSTOM_DVE_SPECS[name] = spec
        dve_ops._SUB_OPCODE_FOR_NAME[name] = opcode
        return op

    def _ref1(in0, in1, s0, s1, imm2):
        return np.square(
            np.maximum(in0.astype(np.float32) * np.float32(s0), np.float32(s1))
        ).astype(np.float32)

    def _ref2(in0, in1, s0, s1, imm2):
        return np.square(
            np.maximum(in0.astype(np.float32) * in1.astype(np.float32), np.float32(s1))
        ).astype(np.float32)

    def _ref3(in0, in1, s0, s1, imm2):
        return (in0.astype(np.float32) * np.float32(s0)).astype(np.float32)

    op1 = _register(
        "CIRCLE_CLAMP_SQ", Spec(body=sq(maxx(Src0 * C0, C1)), reference=_ref1)
    )
    op2 = _register(
        "CIRCLE_CLAMP_SQ2", Spec(body=sq(maxx(Src0 * Src1, C1)), reference=_ref2)
    )
    op3 = _register("CIRCLE_MUL", Spec(body=Src0 * C0, reference=_ref3))
    return op1, op2, op3


def _build():
    if "nc" in _cache:
        return _cache["nc"]
    op_sq, op_sq2, op_mul = _get_custom_ops()
    from concourse.dve_ops import TENSOR_TENSOR_REDUCE as op_ttr
    nc = bacc.Bacc("TRN2", target_bir_lowering=False)

    a_in = nc.declare_dram_parameter("a_shard", [MPC, D], F32, isOutput=False)
    bT_in = nc.declare_dram_parameter("bT16", [D, B], F16, isOutput=False)
    out = nc.declare_dram_parameter("losses", [MPC], F32, isOutput=True)
    ssb_scr = nc.dram_tensor("ssb_scratch", [B], F32)
    ivb_scr = nc.dram_tensor("ivb_scratch", [B], F32)
    out_pm = out.rearrange("(m p) -> p m", p=128)  # [128, 8] view

    with tile.TileContext(nc) as tc:
        with (
            tc.tile_pool(name="consts", bufs=1) as consts,
            tc.tile_pool(name="big", bufs=1) as big,
            tc.tile_pool(name="bsqp", bufs=2) as bsqp,
            tc.tile_pool(name="norm", bufs=2) as normp,
            tc.tile_pool(name="flat", bufs=2) as flatp,
            tc.tile_pool(name="zpool", bufs=2) as zpool,
            tc.tile_pool(name="stats", bufs=1) as stats,
            tc.tile_pool(name="psim", bufs=2, space="PSUM") as psim,
        ):
            # ---- constants ----
            eye = consts.tile([128, 128], F32, tag="eye")
            make_identity(nc, eye)
            ones16 = consts.tile([128, 1], F16, tag="ones16")
            nc.vector.memset(ones16, 1.0)
            onecol = consts.tile([1, 128], F32, tag="onecol")
            nc.vector.memset(onecol, 1.0)
            b_ln16 = consts.tile([128, 1], F32, tag="b_ln16")
            nc.vector.memset(b_ln16, LN16)
            b_m16 = consts.tile([128, 1], F32, tag="b_m16")
            nc.vector.memset(b_m16, -16.0)

            # ---- persistent tensors ----
            bT16 = big.tile([128, B], F16, tag="bT16")  # raw b^T (fp16)
            ivbb = big.tile([128, B], F16, tag="ivbb")  # bcast 1/||b_j||
            aT16 = big.tile([128, MPC], F16, tag="aT16")  # (a*16/||a||)^T
            escr = big.tile([128, B], BF16, tag="escr")  # exp dump (never read)
            S = stats.tile([128, NT], F32, tag="S")  # sum_j exp per a-row
            rdraw = stats.tile([128, NT], F32, tag="rdraw")  # raw diag r
            dscr = stats.tile([128, 128], F32, tag="dscr")  # TTR dummy out

            # ---- a prep: sumsq -> inva16 -> scale(f32) -> transpose -> fp16 ----
            a128 = big.tile([128, NT, D], F32, tag="a128")
            nc.sync.dma_start(out=a128, in_=a_in.rearrange("(t p) d -> p t d", p=128))
            ssa = stats.tile([128, NT], F32, tag="ssa")
            for t in range(NT):
                nc.vector._custom_dve(
                    op_ttr,
                    out=dscr,
                    in0=a128[:, t, :],
                    in1=a128[:, t, :],
                    s0=0.0,
                    s1=1.0,
                    accum_out=ssa[:, t:t + 1],
                )
            lssa = stats.tile([128, NT], F32, tag="lssa")
            nc.scalar.activation(out=lssa, in_=ssa, func=AF.Ln)
            inva16 = stats.tile([128, NT], F32, tag="inva16")
            nc.scalar.activation(
                out=inva16, in_=lssa, func=AF.Exp, scale=-0.5, bias=b_ln16
            )
            a16 = big.tile([128, NT, D], F32, tag="a16")
            for t in range(NT):
                nc.vector._custom_dve(
                    op_mul,
                    out=a16[:, t, :],
                    in0=a128[:, t, :],
                    s0=inva16[:, t:t + 1],
                )
            for q in range(2):  # two psum batches of 4 transposes
                pt = psim.tile([128, CHUNK], F32, tag="sim")
                for j in range(4):
                    nc.tensor.transpose(
                        pt[:, j * 128:(j + 1) * 128], a16[:, q * 4 + j, :], eye
                    )
                nc.scalar.copy(out=aT16[:, q * 512:(q + 1) * 512], in_=pt[:, 0:512])

            # ---- b prep, batched ----
            # Phase A: load bT16, square, ones-matmul column sums -> ssb_scr
            ssbf_pool = flatp
            for j in range(NK // 2):  # 4 psum tiles, 2 chunks each
                K = j * 2048
                for kk in range(2):
                    Kc = K + kk * 1024
                    nc.gpsimd.dma_start(
                        out=bT16[:, Kc:Kc + 1024], in_=bT_in[:, Kc:Kc + 1024]
                    )
                    bsq = bsqp.tile([128, 1024], F16, tag="bsq")
                    nc.vector.tensor_mul(
                        bsq, bT16[:, Kc:Kc + 1024], bT16[:, Kc:Kc + 1024]
                    )
                    if kk == 0:
                        pt = psim.tile([128, CHUNK], F32, tag="sim")
                    for h in range(2):
                        nc.tensor.matmul(
                            pt[0:1, kk * 1024 + h * 512:kk * 1024 + (h + 1) * 512],
                            ones16,
                            bsq[:, h * 512:(h + 1) * 512],
                            start=True,
                            stop=True,
                        )
                ssbf = flatp.tile([1, 2048], F32, tag="ssbf")
                nc.scalar.copy(out=ssbf, in_=pt[0:1, 0:2048])
                nc.sync.dma_start(out=ssb_scr[K:K + 2048], in_=ssbf)
            # Phase B: one reshape DMA, one Ln, one Exp, one scatter, one load
            ssb_all = stats.tile([128, NK * 8], F32, tag="ssb_all")
            nc.sync.dma_start(
                out=ssb_all, in_=ssb_scr.rearrange("(m p) -> p m", p=128)
            )
            lssb = stats.tile([128, NK * 8], F32, tag="lssb")
            nc.scalar.activation(out=lssb, in_=ssb_all, func=AF.Ln)
            ivb_all = stats.tile([128, NK * 8], F32, tag="ivb_all")
            nc.scalar.activation(out=ivb_all, in_=lssb, func=AF.Exp, scale=-0.5)
            nc.sync.dma_start(
                out=ivb_scr.rearrange("(m p) -> p m", p=128), in_=ivb_all
            )
            ivbf_all = big.tile([1, B], F32, tag="ivbf_all")
            nc.sync.dma_start(out=ivbf_all, in_=ivb_scr[0:B])
            # Phase C: broadcast to [128, B] via K=1 ones-matmul + ACT copy
            for k in range(NK):
                K = k * 1024
                ptb = psim.tile([128, CHUNK], F32, tag="sim")
                for h in range(2):
                    nc.tensor.matmul(
                        ptb[:, h * 512:(h + 1) * 512],
                        onecol,
                        ivbf_all[:, K + h * 512:K + (h + 1) * 512],
                        start=True,
                        stop=True,
                    )
                nc.scalar.copy(out=ivbb[:, K:K + 1024], in_=ptb[:, 0:1024])

            # ---- main: per a-tile t, full [128 x 8192] row ----
            for t in range(0 if _dbg("skipmain") else NT):
                zb = zpool.tile([128, B], F32, tag="z")
                for c in range(NC):
                    cc = c * CHUNK
                    pt = psim.tile([128, CHUNK], F32, tag="sim")
                    for h in range(CHUNK // 512):
                        nc.tensor.matmul(
                            pt[:, h * 512:(h + 1) * 512],
                            aT16[:, t * 128:(t + 1) * 128],
                            bT16[:, cc + h * 512:cc + (h + 1) * 512],
                            start=True,
                            stop=True,
                        )
                    if c == 0:
                        nc.vector._custom_dve(
                            op_ttr,
                            out=dscr,
                            in0=pt[:, t * 128:(t + 1) * 128],
                            in1=eye,
                            s0=0.0,
                            s1=1.0,
                            accum_out=rdraw[:, t:t + 1],
                        )
                    if _dbg("noop2"):
                        nc.vector._custom_dve(
                            op_sq,
                            out=zb[:, cc:cc + CHUNK],
                            in0=pt,
                            s0=0.0884,
                            s1=4.0,
                        )
                    else:
                        nc.vector._custom_dve(
                            op_sq2,
                            out=zb[:, cc:cc + CHUNK],
                            in0=pt,
                            in1=ivbb[:, cc:cc + CHUNK],
                            s1=4.0,
                        )
                if _dbg("noaccum"):
                    nc.scalar.activation(out=escr, in_=zb, func=AF.Exp, bias=b_m16)
                    nc.vector.tensor_reduce(
                        out=S[:, t:t + 1],
                        in_=escr,
                        axis=mybir.AxisListType.X,
                        op=OP.add,
                    )
                else:
                    nc.scalar.activation(
                        out=escr,
                        in_=zb,
                        func=AF.Exp,
                        bias=b_m16,
                        accum_out=S[:, t:t + 1],
                    )

            # ---- epilogue: [128, 8] ----
            if _dbg("skipmain"):
                nc.vector.memset(S, 1.0)
                nc.vector.memset(rdraw, 1.0)
            rd16 = stats.tile([128, NT], F32, tag="rd16")  # 16 * s_ii
            nc.vector.tensor_mul(rd16, rdraw, ivb_all[:, 0:NT])
            zd = stats.tile([128, NT], F32, tag="zd")
            nc.vector._custom_dve(op_sq, out=zd, in0=rd16, s0=1.0, s1=4.0)
            ed = stats.tile([128, NT], F32, tag="ed")
            nc.scalar.activation(out=ed, in_=zd, func=AF.Exp, bias=b_m16)
            S2 = stats.tile([128, NT], F32, tag="S2")
            nc.vector.scalar_tensor_tensor(
                out=S2, in0=S, scalar=0.0, in1=ed, op0=OP.add, op1=OP.subtract
            )
            lse = stats.tile([128, NT], F32, tag="lse")
            nc.scalar.activation(out=lse, in_=S2, func=AF.Ln)
            w = stats.tile([128, NT], F32, tag="w")
            nc.vector.scalar_tensor_tensor(
                out=w, in0=rd16, scalar=12.0, in1=rd16, op0=OP.min, op1=OP.min
            )
            lpr = stats.tile([128, NT], F32, tag="lpr")
            nc.vector.scalar_tensor_tensor(
                out=lpr, in0=w, scalar=16.0, in1=w, op0=OP.subtract, op1=OP.mult
            )
            tt = stats.tile([128, NT], F32, tag="tt")
            nc.vector.scalar_tensor_tensor(
                out=tt, in0=lpr, scalar=48.0, in1=lse, op0=OP.add, op1=OP.add
            )
            abst = stats.tile([128, NT], F32, tag="abst")
            nc.scalar.activation(out=abst, in_=tt, func=AF.Abs)
            u = stats.tile([128, NT], F32, tag="u")
            nc.scalar.activation(out=u, in_=abst, func=AF.Exp, scale=-1.0)
            v = stats.tile([128, NT], F32, tag="v")
            nc.scalar.activation(out=v, in_=u, func=AF.Ln, bias=1.0)
            loss = stats.tile([128, NT], F32, tag="loss")
            nc.vector.scalar_tensor_tensor(
                out=loss, in0=tt, scalar=0.0, in1=v, op0=OP.max, op1=OP.add
            )
            nc.sync.dma_start(out=out_pm, in_=loss)

    nc.finalize()
    _cache["nc"] = nc
    return nc


def _make_in_maps(A: np.ndarray, Bm: np.ndarray) -> list[dict]:
    in_maps = []
    for c in range(NCORES):
        br = np.roll(Bm, -MPC * c, axis=0)
        in_maps.append(
            {
                "a_shard": np.ascontiguousarray(A[MPC * c:MPC * (c + 1)]),
                "bT16": np.ascontiguousarray(br.T).astype(np.float16),
            }
        )
    return in_maps


def kernel(embeddings_a: np.ndarray, embeddings_b: np.ndarray) -> np.ndarray:
    nc = _build()
    A = np.ascontiguousarray(embeddings_a, dtype=np.float32)
    Bm = np.ascontiguousarray(embeddings_b, dtype=np.float32)
    in_maps = _make_in_maps(A, Bm)
    res = run_bass_kernel_spmd(nc, in_maps, list(range(NCORES))).results
    losses = np.concatenate([res[c]["losses"] for c in range(NCORES)])
    return np.float32(np.mean(losses.astype(np.float64)))


# revision 19
# speedup vs baseline: 2.1374x; 1.3671x over previous
"""CircleLoss (nn_CircleLoss_55482387529741) Trainium2 Bass kernel, v2.

Math (B=8192, D=128, m=0.25, g=256):
  ahat = l2norm(A) rows, bhat = l2norm(B) rows, s_ij = ahat_i . bhat_j
  exp(logit_neg) = exp(max(16*s, 4)^2 - 16)   (identity; cold s<=m -> exp(0)=1)
  lse_pos_ii = (w-12)(w-4), w = min(16*s_ii, 12)
  loss_i = softplus(lse_pos_i + log(sum_{j!=i} exp(logit_neg_ij)));  out = mean

Distribution: A rows sharded 8 x 1024.  Core layout is FLIPPED vs v1:
partitions = a-rows (8 tiles of 128), free = all 8192 b-cols.  The row
reduction sum_j exp(...) then rides the ACT engine's fused accum_out,
eliminating the ones-matmul reduction pass entirely.  B is rotated per
core on host so each a-tile t's diagonal lands at b-cols [t*128, t*128+128).

Per core engine pipeline (steady state):
  PE    : sim matmuls  aT16[t] (stationary, fp16) x bT16 chunks (moving, fp16)
  DVE   : z = sq(max(r * invb_j, 4))  custom 2-src op, PSUM -> SBUF f32
  ACT   : e = exp(z - 16) with accum_out = per-row sums  (one [128,8192] op / tile)
  GPSIMD: b sumsq squares, invb partition-broadcast (prep), off hot path
Norms: ssb via ones-matmul on PE (column sums of bsq) -> DRAM round-trip
reshape -> rsqrt via ln/exp -> broadcast [1,8192] -> [128,8192] (gpsimd).
Diagonal excluded by subtracting exp(logit_neg(s_ii)) from the row sum.
"""

import sys

for _p in ("/opt/trn_rl_repo",):
    if _p not in sys.path:
        sys.path.append(_p)

import numpy as np

import concourse.bass as bass
from concourse import bacc
import concourse.mybir as mybir
import concourse.tile as tile
from concourse.bass_utils import run_bass_kernel_spmd
from concourse.masks import make_identity

F32 = mybir.dt.float32
F16 = mybir.dt.float16
BF16 = mybir.dt.bfloat16
AF = mybir.ActivationFunctionType
OP = mybir.AluOpType

B = 8192
D = 128
NCORES = 8
MPC = B // NCORES  # 1024 a-rows per core
NT = MPC // 128  # 8 a-tiles
NK = B // 1024  # 8 b-chunks of 1024
CHUNK = 2048  # z-op granularity (psum tile free size)
NC = B // CHUNK  # 4 psum chunks per a-tile row
LN16 = float(np.log(16.0))

_cache = {}

import os


def _dbg(flag):
    return flag in os.environ.get("KDBG", "").split(",")


def _get_custom_ops():
    """Register (once) the custom DVE ops:
    CIRCLE_CLAMP_SQ : out = sq(maxx(in0*s0, s1))   (scalar-scale variant)
    CIRCLE_CLAMP_SQ2: out = sq(maxx(in0*in1, s1))  (2-src: in1 = invb bcast)
    """
    from concourse import dve_ops
    from concourse.dve_spec import Spec, Src0, Src1, C0, C1, maxx, sq, lower
    from concourse.dve_spec import _has_src1 as has_src1
    from concourse.dve_uop import DveOpSpec

    def _register(name, spec):
        for o in dve_ops.OPS:
            if o.name == name:
                return o
        opcode = dve_ops._CUSTOM_DVE_ROW_BASE + len(dve_ops.OPS)
        assert opcode < 0x20
        shas = {}
        for ver in ("v3", "v4"):
            try:
                shas[ver] = DveOpSpec(
                    name=name,
                    opcode=opcode,
                    uops=lower(spec, ver=ver),
                    rd1_en=has_src1(spec),
                ).sha(ver)
            except Exception:
                pass
        op = dve_ops.DveOp(name, spec, subdim=False, uops_sha=shas)
        dve_ops.OPS.append(op)
        dve_ops.CUSTOM_DVE_SPECS[name] = spec
        dve_ops._SUB_OPCODE_FOR_NAME[name] = opcode
        return op

    def _ref1(in0, in1, s0, s1, imm2):
        return np.square(
            np.maximum(in0.astype(np.float32) * np.float32(s0), np.float32(s1))
        ).astype(np.float32)

    def _ref2(in0, in1, s0, s1, imm2):
        return np.square(
            np.maximum(in0.astype(np.float32) * in1.astype(np.float32), np.float32(s1))
        ).astype(np.float32)

    def _ref3(in0, in1, s0, s1, imm2):
        return (in0.astype(np.float32) * np.float32(s0)).astype(np.float32)

    op1 = _register(
        "CIRCLE_CLAMP_SQ", Spec(body=sq(maxx(Src0 * C0, C1)), reference=_ref1)
    )
    op2 = _register(
        "CIRCLE_CLAMP_SQ2", Spec(body=sq(maxx(Src0 * Src1, C1)), reference=_ref2)
    )
    op3 = _register("CIRCLE_MUL", Spec(body=Src0 * C0, reference=_ref3))
    return op1, op2, op3


def _patch_act_tables():
    """Prefer the combined ln+exp ACT table so Ln/Exp sequences don't
    thrash ACT_TABLE_LOADs (1.28us each)."""
    # Reordering get_activation_tables changes act_func_set_id indices and
    # desyncs from walrus's own act_info.json mapping -> engine crash.
    # Left disabled; Ln/Exp batching keeps table loads low instead.
    return


def _build():
    if "nc" in _cache:
        return _cache["nc"]
    _patch_act_tables()
    op_sq, op_sq2, op_mul = _get_custom_ops()
    from concourse.dve_ops import TENSOR_TENSOR_REDUCE as op_ttr
    nc = bacc.Bacc("TRN2", target_bir_lowering=False)

    a_in = nc.declare_dram_parameter("a_shard", [MPC, D], F32, isOutput=False)
    bT_in = nc.declare_dram_parameter("bT16", [D, B], F16, isOutput=False)
    out = nc.declare_dram_parameter("losses", [MPC], F32, isOutput=True)
    out_pm = out.rearrange("(p m) -> p m", m=NT)  # [128, 8] view, p-major

    with tile.TileContext(nc) as tc:
        with (
            tc.tile_pool(name="consts", bufs=1) as consts,
            tc.tile_pool(name="big", bufs=1) as big,
            tc.tile_pool(name="bsqp", bufs=2) as bsqp,
            tc.tile_pool(name="norm", bufs=2) as normp,
            tc.tile_pool(name="flat", bufs=2) as flatp,
            tc.tile_pool(name="zpool", bufs=2) as zpool,
            tc.tile_pool(name="stats", bufs=1) as stats,
            tc.tile_pool(name="psim", bufs=2, space="PSUM") as psim,
        ):
            # ---- constants ----
            eye = consts.tile([128, 128], F32, tag="eye")
            make_identity(nc, eye)
            ones16 = consts.tile([128, 1], F16, tag="ones16")
            nc.vector.memset(ones16, 1.0)
            onecol = consts.tile([1, 128], F32, tag="onecol")
            nc.vector.memset(onecol, 1.0)
            b_ln16 = consts.tile([128, 1], F32, tag="b_ln16")
            nc.vector.memset(b_ln16, LN16)
            b_m16 = consts.tile([128, 1], F32, tag="b_m16")
            nc.vector.memset(b_m16, -16.0)

            # ---- persistent tensors ----
            bT16 = big.tile([128, B], F16, tag="bT16")  # raw b^T (fp16)
            ssbflat = big.tile([1, B], F32, tag="ssbflat")  # ||b_j||^2, j-order
            lssbB = big.tile([128, B], F32, tag="lssbB")  # ln(ssb) bcast
            ivbb = big.tile([128, B], F16, tag="ivbb")  # bcast 1/||b_j||
            aT16 = big.tile([128, MPC], F16, tag="aT16")  # (a*16/||a||)^T
            escr = big.tile([128, B], BF16, tag="escr")  # exp dump (never read)
            S = stats.tile([128, NT], F32, tag="S")  # sum_j exp per a-row
            rdraw = stats.tile([128, NT], F32, tag="rdraw")  # raw diag r
            dscr = stats.tile([128, 128], F32, tag="dscr")  # TTR dummy out

            # ---- a prep: sumsq -> inva16 -> scale(f32) -> transpose -> fp16 ----
            a128 = big.tile([128, NT, D], F32, tag="a128")
            nc.sync.dma_start(out=a128, in_=a_in.rearrange("(t p) d -> p t d", p=128))
            ssa = stats.tile([128, NT], F32, tag="ssa")
            for t in range(NT):
                nc.vector._custom_dve(
                    op_ttr,
                    out=dscr,
                    in0=a128[:, t, :],
                    in1=a128[:, t, :],
                    s0=0.0,
                    s1=1.0,
                    accum_out=ssa[:, t:t + 1],
                )
            lssa = stats.tile([128, NT], F32, tag="lssa")
            nc.scalar.activation(out=lssa, in_=ssa, func=AF.Ln)
            inva16 = stats.tile([128, NT], F32, tag="inva16")
            nc.scalar.activation(
                out=inva16, in_=lssa, func=AF.Exp, scale=-0.5, bias=b_ln16
            )
            a16 = big.tile([128, NT, D], F32, tag="a16")
            for t in range(NT):
                nc.vector._custom_dve(
                    op_mul,
                    out=a16[:, t, :],
                    in0=a128[:, t, :],
                    s0=inva16[:, t:t + 1],
                )
            for q in range(2):  # two psum batches of 4 transposes
                pt = psim.tile([128, CHUNK], F32, tag="sim")
                for j in range(4):
                    nc.tensor.transpose(
                        pt[:, j * 128:(j + 1) * 128], a16[:, q * 4 + j, :], eye
                    )
                nc.scalar.copy(out=aT16[:, q * 512:(q + 1) * 512], in_=pt[:, 0:512])

            # ---- b prep: all on-chip, batched by engine ----
            # Phase A: load bT16, square (DVE), ones-matmul col sums (PE),
            # copy psum -> ssbflat [1, B] (ACT).
            for j in range(NK // 2):  # 4 psum tiles, 2 chunks each
                K = j * 2048
                for kk in range(2):
                    Kc = K + kk * 1024
                    nc.gpsimd.dma_start(
                        out=bT16[:, Kc:Kc + 1024], in_=bT_in[:, Kc:Kc + 1024]
                    )
                    bsq = bsqp.tile([128, 1024], F16, tag="bsq")
                    nc.vector.tensor_mul(
                        bsq, bT16[:, Kc:Kc + 1024], bT16[:, Kc:Kc + 1024]
                    )
                    if kk == 0:
                        pt = psim.tile([128, CHUNK], F32, tag="sim")
                    for h in range(2):
                        nc.tensor.matmul(
                            pt[0:1, kk * 1024 + h * 512:kk * 1024 + (h + 1) * 512],
                            ones16,
                            bsq[:, h * 512:(h + 1) * 512],
                            start=True,
                            stop=True,
                        )
                nc.scalar.copy(out=ssbflat[0:1, K:K + 2048], in_=pt[0:1, 0:2048])
            # Phase C: bcast raw ssb via K=1 ones-matmul, Ln batch, Exp batch.
            for k in range(NK):
                K = k * 1024
                ptb = psim.tile([128, CHUNK], F32, tag="sim")
                for h in range(2):
                    nc.tensor.matmul(
                        ptb[:, h * 512:(h + 1) * 512],
                        onecol,
                        ssbflat[0:1, K + h * 512:K + (h + 1) * 512],
                        start=True,
                        stop=True,
                    )
                nc.scalar.activation(
                    out=lssbB[:, K:K + 1024], in_=ptb[:, 0:1024], func=AF.Ln
                )
            for k in range(NK):
                K = k * 1024
                nc.scalar.activation(
                    out=ivbb[:, K:K + 1024],
                    in_=lssbB[:, K:K + 1024],
                    func=AF.Exp,
                    scale=-0.5,
                )

            # ---- main: per a-tile t, full [128 x 8192] row ----
            for t in range(0 if _dbg("skipmain") else NT):
                zb = zpool.tile([128, B], F32, tag="z")
                for c in range(NC):
                    cc = c * CHUNK
                    pt = psim.tile([128, CHUNK], F32, tag="sim")
                    for h in range(CHUNK // 512):
                        nc.tensor.matmul(
                            pt[:, h * 512:(h + 1) * 512],
                            aT16[:, t * 128:(t + 1) * 128],
                            bT16[:, cc + h * 512:cc + (h + 1) * 512],
                            start=True,
                            stop=True,
                        )
                    if c == 0:
                        nc.vector._custom_dve(
                            op_ttr,
                            out=dscr,
                            in0=pt[:, t * 128:(t + 1) * 128],
                            in1=eye,
                            s0=0.0,
                            s1=1.0,
                            accum_out=rdraw[:, t:t + 1],
                        )
                    if _dbg("noop2"):
                        nc.vector._custom_dve(
                            op_sq,
                            out=zb[:, cc:cc + CHUNK],
                            in0=pt,
                            s0=0.0884,
                            s1=4.0,
                        )
                    else:
                        nc.vector._custom_dve(
                            op_sq2,
                            out=zb[:, cc:cc + CHUNK],
                            in0=pt,
                            in1=ivbb[:, cc:cc + CHUNK],
                            s1=4.0,
                        )
                if _dbg("noaccum"):
                    nc.scalar.activation(out=escr, in_=zb, func=AF.Exp, bias=b_m16)
                    nc.vector.tensor_reduce(
                        out=S[:, t:t + 1],
                        in_=escr,
                        axis=mybir.AxisListType.X,
                        op=OP.add,
                    )
                else:
                    nc.scalar.activation(
                        out=escr,
                        in_=zb,
                        func=AF.Exp,
                        bias=b_m16,
                        accum_out=S[:, t:t + 1],
                    )

            # ---- epilogue: [128, 8] ----
            if _dbg("skipmain"):
                nc.vector.memset(S, 1.0)
                nc.vector.memset(rdraw, 1.0)
            lsd = stats.tile([128, NT], F32, tag="lsd")  # ln ssb at diag
            for t in range(NT):
                nc.vector._custom_dve(
                    op_ttr,
                    out=dscr,
                    in0=lssbB[:, t * 128:(t + 1) * 128],
                    in1=eye,
                    s0=0.0,
                    s1=1.0,
                    accum_out=lsd[:, t:t + 1],
                )
            ivbd = stats.tile([128, NT], F32, tag="ivbd")
            nc.scalar.activation(out=ivbd, in_=lsd, func=AF.Exp, scale=-0.5)
            rd16 = stats.tile([128, NT], F32, tag="rd16")  # 16 * s_ii
            nc.vector.tensor_mul(rd16, rdraw, ivbd)
            zd = stats.tile([128, NT], F32, tag="zd")
            nc.vector._custom_dve(op_sq, out=zd, in0=rd16, s0=1.0, s1=4.0)
            ed = stats.tile([128, NT], F32, tag="ed")
            nc.scalar.activation(out=ed, in_=zd, func=AF.Exp, bias=b_m16)
            S2 = stats.tile([128, NT], F32, tag="S2")
            nc.vector.scalar_tensor_tensor(
                out=S2, in0=S, scalar=0.0, in1=ed, op0=OP.add, op1=OP.subtract
            )
            lse = stats.tile([128, NT], F32, tag="lse")
            nc.scalar.activation(out=lse, in_=S2, func=AF.Ln)
            w = stats.tile([128, NT], F32, tag="w")
            nc.vector.scalar_tensor_tensor(
                out=w, in0=rd16, scalar=12.0, in1=rd16, op0=OP.min, op1=OP.min
            )
            lpr = stats.tile([128, NT], F32, tag="lpr")
            nc.vector.scalar_tensor_tensor(
                out=lpr, in0=w, scalar=16.0, in1=w, op0=OP.subtract, op1=OP.mult
            )
            tt = stats.tile([128, NT], F32, tag="tt")
            nc.vector.scalar_tensor_tensor(
                out=tt, in0=lpr, scalar=48.0, in1=lse, op0=OP.add, op1=OP.add
            )
            abst = stats.tile([128, NT], F32, tag="abst")
            nc.scalar.activation(out=abst, in_=tt, func=AF.Abs)
            u = stats.tile([128, NT], F32, tag="u")
            nc.scalar.activation(out=u, in_=abst, func=AF.Exp, scale=-1.0)
            v = stats.tile([128, NT], F32, tag="v")
            nc.scalar.activation(out=v, in_=u, func=AF.Ln, bias=1.0)
            loss = stats.tile([128, NT], F32, tag="loss")
            nc.vector.scalar_tensor_tensor(
                out=loss, in0=tt, scalar=0.0, in1=v, op0=OP.max, op1=OP.add
            )
            nc.sync.dma_start(out=out_pm, in_=loss)

    nc.finalize()
    _cache["nc"] = nc
    return nc


def _make_in_maps(A: np.ndarray, Bm: np.ndarray) -> list[dict]:
    in_maps = []
    for c in range(NCORES):
        br = np.roll(Bm, -MPC * c, axis=0)
        in_maps.append(
            {
                "a_shard": np.ascontiguousarray(A[MPC * c:MPC * (c + 1)]),
                "bT16": np.ascontiguousarray(br.T).astype(np.float16),
            }
        )
    return in_maps


def kernel(embeddings_a: np.ndarray, embeddings_b: np.ndarray) -> np.ndarray:
    nc = _build()
    A = np.ascontiguousarray(embeddings_a, dtype=np.float32)
    Bm = np.ascontiguousarray(embeddings_b, dtype=np.float32)
    in_maps = _make_in_maps(A, Bm)
    res = run_bass_kernel_spmd(nc, in_maps, list(range(NCORES))).results
    losses = np.concatenate([res[c]["losses"] for c in range(NCORES)])
    return np.float32(np.mean(losses.astype(np.float64)))
